# revision 84
# baseline (speedup 1.0000x reference)
"""Trainium2 Bass kernel for nn_Attention_57827439673725.

Dense transformer attention block (B=32, N=1024, C=1024, H=16, hd=64):
  qkv = x @ qkv_w + qkv_b ; q,k rms-normed (per head) and 2D-roped;
  out = softmax(q k^T / sqrt(hd)) v @ proj_w + proj_b

Pure data-parallel over batch across 8 NeuronCores (4 batches each).

v5 = v3 numerics + v4 scheduling. The v4 fp8-attention experiment (fp8
q/k/P/v, DoubleRow S and O) measured 6.6% rel err: with random data the
attention output shrinks as fast as independent noise averages (O is a
random-walk mean), so every fp8 tensor costs its raw ~2.5-4% relative -
each individually over the 2e-2 gate. bf16 attention it is.

Per-core design:
  phase A (per token tile): qkv matmul in fp8e4 DoubleRow mode as a
        3-chain residual product (x_hi+x_lo)(W_hi+W_lo) minus the lo*lo
        term, W pre-scaled by 32 to clear fp8 subnormals (rms-norm makes
        q/k scale-invariant; 1/32 is folded into proj_w for the v path).
        qkv psum -> SBUF bf16 by ONE ACT copy; rms stats on DVE (bf16
        squares, 2 folds + 16-wide reduce since TensorReduce gets no 2x
        mode, bf16-bits ln trick + Newton rsqrt); rope multiplies on DVE;
        the rope ADD and the rsqrt multiply run on the otherwise-idle Pool
        engine (rope commutes with the per-(tok,head) rms scale, so the
        rsqrt multiply fuses after the add). q|k rope bf16 [128, 2048] ->
        one XBAR transpose -> head-major qkT; v lands via one ACT copy.
  phase B (per head h, per j-tile): S^T = k q^T (bf16, K=64, row base
        64*(h%2) via tile_position); ONE ACT table exp over the full
        [128 j, 1024 i] psum -> P^T bf16 (emitted one slot late so it
        never waits at the ACT queue head); O in natural layout: P^T
        block stationary, v65 column-block moving, 64 moving cols per
        128x64 output; parallel 1-col matmuls against a ones column
        accumulate the softmax denominator (single psum group per bank).
        normalize = per-partition DVE reciprocal + one broadcast-last
        multiply into attn4.
  phase C: attn4 -> 8 upfront XBAR transposes (no waits, so they clear
        the SP queue before the rope-gated qkT transposes) -> proj bf16
        -> DVE copy -> DMA out fp32 on the ACT queue (its only wait
        precedes it there).

Slot pipeline: phase B is a slot pipeline; a weave generator (C(b-1)
first, then A(b+1), front-loaded 2-steps/slot early on) yields chunks of
PE work between slots so PE stays busy and p-state-ramped through the
exp-bound stretches. PSUM: shared pool of 3x [128,1024] f32 (S slots,
qkv tiles, proj tiles) + psO [128, NT, 64] + psD [128, H, NT] = 8 banks.
"""

import os
import sys

import numpy as np

for _p in ("/opt/trn_rl_repo",):
    if os.path.isdir(_p) and _p not in sys.path:
        sys.path.insert(0, _p)

import ml_dtypes  # noqa: E402

import concourse.bass as bass  # noqa: E402
import concourse.mybir as mybir  # noqa: E402
import concourse.tile as tile  # noqa: E402
from concourse import bacc  # noqa: E402
from concourse.bass_utils import run_bass_kernel_spmd  # noqa: E402

BF16 = mybir.dt.bfloat16
F32 = mybir.dt.float32
F8 = mybir.dt.float8e4
NPBF16 = ml_dtypes.bfloat16
NPF8 = ml_dtypes.float8_e4m3fn

N_CORES = 8
B, N, C = 32, 1024, 1024
H, HD = 16, 64
BSH = B // N_CORES  # batches per core
NT = N // 128  # token tiles per batch
KT = C // 128  # k tiles over C
KP = KT // 2  # fp8 DoubleRow k-pair count
EPS = 1e-06
THETA = 10000.0
WSCALE = 32.0  # qkv_w prescale (clears fp8e4m3 subnormals)

MULT = mybir.AluOpType.mult
ADD = mybir.AluOpType.add
DR = mybir.MatmulPerfMode.DoubleRow
CHUNK = int(os.environ.get("KCHUNK", "4"))
PROLOG = int(os.environ.get("KPROLOG", "2"))
O_DELAY = int(os.environ.get("KODELAY", "2"))  # in jt slots
KPOOL = int(os.environ.get("KPOOL", "1"))  # 1: rope add+rr-mul on Pool


def _ap_with(ap: bass.AP, dims) -> bass.AP:
    return bass.AP(tensor=ap.tensor, offset=ap.offset, ap=dims)


def _bcast_mid(ap: bass.AP, n: int) -> bass.AP:
    """[P, F] -> [P, n, F] with a 0-step broadcast middle dim."""
    return _ap_with(ap, [ap.ap[0], [0, n], *ap.ap[1:]])


def _bcast_last(ap: bass.AP, n: int) -> bass.AP:
    """[P, F] -> [P, F, n] with a 0-step broadcast last dim."""
    return _ap_with(ap, [*ap.ap, [0, n]])


def _build_module(use_bias: bool):
    nc = bacc.Bacc(
        "TRN2", target_bir_lowering=False, debug=False,
        dynamic_dma_scratch_size=2048,
    )

    xhi_d = nc.dram_tensor("xhi", [BSH, KT, 128, N], F8, kind="ExternalInput")
    xlo_d = nc.dram_tensor("xlo", [BSH, KT, 128, N], F8, kind="ExternalInput")
    whi_d = nc.dram_tensor("whi", [KT, 128, 3 * C], F8, kind="ExternalInput")
    wlo_d = nc.dram_tensor("wlo", [KT, 128, 3 * C], F8, kind="ExternalInput")
    wproj_d = nc.dram_tensor("wproj", [KT, 128, C], BF16, kind="ExternalInput")
    tabs_d = nc.dram_tensor("tabs", [4, NT, 128, HD], BF16, kind="ExternalInput")
    if use_bias:
        bq_d = nc.dram_tensor("bq", [3 * C], BF16, kind="ExternalInput")  # *WSCALE
        bp_d = nc.dram_tensor("bp", [C], BF16, kind="ExternalInput")
    out_d = nc.dram_tensor("out", [BSH, NT, 128, C], F32, kind="ExternalOutput")

    from contextlib import ExitStack

    with ExitStack() as ctx:
        tc = ctx.enter_context(tile.TileContext(nc))
        pool = lambda name, bufs, **kw: ctx.enter_context(  # noqa: E731
            tc.tile_pool(name=name, bufs=bufs, **kw)
        )
        cfg = dict(
            qk=2, v65=2, pt=2, at=2, qs=2, t1=2, t2=2, stats=2,
            rec=2, outs=1, a4=1, psS=3, psQ=0, psO=1,
        )
        for kv in os.environ.get("KBUFS", "").split(","):
            if kv:
                kk, vv = kv.split("=")
                cfg[kk] = int(vv)

        wpool = pool("weights", 1)
        cpool = pool("consts", 1)
        xtpool = pool("xt", 1)
        qkpool = pool("qkT", cfg["qk"])
        vpool = pool("v65", cfg["v65"])
        ptpool = pool("pt", cfg["pt"])
        a4pool = pool("attn4", cfg["a4"])
        atpool = pool("attnT", cfg["at"])
        qspool = pool("qs", cfg["qs"])
        t1pool = pool("t1", cfg["t1"])
        t2pool = pool("t2", cfg["t2"])
        spool = pool("stats", cfg["stats"])
        recpool = pool("rec", cfg["rec"])
        opool = pool("outs", cfg["outs"])
        psS = pool("psS", cfg["psS"], space="PSUM")
        psQ = pool("psQ", cfg["psQ"], space="PSUM") if cfg["psQ"] else psS
        psO2 = pool("psO", cfg["psO"], space="PSUM")
        psDp = pool("psD", 1, space="PSUM")

        # ---- persistent weights / constants ----
        whi = wpool.tile([128, KT, 3 * C], F8, tag="whi")
        wlo = wpool.tile([128, KT, 3 * C], F8, tag="wlo")
        wproj = wpool.tile([128, KT, C], BF16, tag="wproj")

        tabs = cpool.tile([128, 4, NT, HD], BF16, tag="tabs")
        for i in range(4):
            nc.sync.dma_start(
                out=tabs[:, i, :, :], in_=tabs_d[i].rearrange("t p d -> p t d")
            )
        if use_bias:
            bias_qkv = cpool.tile([128, 3 * C], BF16, tag="bq")
            bq_ap = bq_d[:]
            nc.sync.dma_start(
                out=bias_qkv[:, :], in_=_ap_with(bq_ap, [[0, 128], *bq_ap.ap])
            )
            bias_proj = cpool.tile([128, C], BF16, tag="bp")
            bp_ap = bp_d[:]
            nc.sync.dma_start(
                out=bias_proj[:, :], in_=_ap_with(bp_ap, [[0, 128], *bp_ap.ap])
            )
        ones_col = cpool.tile([128, 1], BF16, tag="ones1")
        nc.vector.memset(ones_col[:, :], 1.0)

        def load_x(b):
            xhi = xtpool.tile([128, KT, N], F8, tag="xhi", name="xhi")
            nc.sync.dma_start(
                out=xhi[:, :, :], in_=xhi_d[b].rearrange("k p n -> p k n")
            )
            xlo = xtpool.tile([128, KT, N], F8, tag="xlo", name="xlo")
            nc.sync.dma_start(
                out=xlo[:, :, :], in_=xlo_d[b].rearrange("k p n -> p k n")
            )
            return xhi, xlo

        def load_weights(b0):
            # first-consumed slices first: the opening q-chain needs only
            # whi/wlo cols 0:512 and x tile 0 - land those in small DMAs so
            # PE starts early, then stream the rest
            nc.sync.dma_start(
                out=whi[:, :, 0:512],
                in_=whi_d[:, :, 0:512].rearrange("k p n -> p k n"),
            )
            xhi = xtpool.tile([128, KT, N], F8, tag="xhi", name="xhi")
            nc.sync.dma_start(
                out=xhi[:, :, 0:128], in_=xhi_d[b0, :, :, 0:128].rearrange("k p n -> p k n")
            )
            nc.sync.dma_start(
                out=wlo[:, :, 0:512],
                in_=wlo_d[:, :, 0:512].rearrange("k p n -> p k n"),
            )
            xlo = xtpool.tile([128, KT, N], F8, tag="xlo", name="xlo")
            nc.sync.dma_start(
                out=xlo[:, :, 0:128], in_=xlo_d[b0, :, :, 0:128].rearrange("k p n -> p k n")
            )
            nc.sync.dma_start(
                out=whi[:, :, 512:1024],
                in_=whi_d[:, :, 512:1024].rearrange("k p n -> p k n"),
            )
            nc.sync.dma_start(
                out=wlo[:, :, 512:1024],
                in_=wlo_d[:, :, 512:1024].rearrange("k p n -> p k n"),
            )
            nc.sync.dma_start(
                out=xhi[:, :, 128:], in_=xhi_d[b0, :, :, 128:].rearrange("k p n -> p k n")
            )
            nc.sync.dma_start(
                out=xlo[:, :, 128:], in_=xlo_d[b0, :, :, 128:].rearrange("k p n -> p k n")
            )
            nc.sync.dma_start(
                out=whi[:, :, 1024:],
                in_=whi_d[:, :, 1024:].rearrange("k p n -> p k n"),
            )
            nc.sync.dma_start(
                out=wlo[:, :, 1024:],
                in_=wlo_d[:, :, 1024:].rearrange("k p n -> p k n"),
            )
            nc.sync.dma_start(
                out=wproj[:, :, :], in_=wproj_d[:, :, :].rearrange("k p n -> p k n")
            )
            return xhi, xlo

        def qkv_mm(ps, xts, col_lo, chunk=None):
            """3-chain fp8 DR product into ps[:, 0:1024].

            Generator when chunk is set: yields between groups of `chunk`
            matmuls so the caller can interleave B-phase slots.
            """
            xhi, xlo = xts
            chains = [(xhi, whi), (xhi, wlo), (xlo, whi)]
            nch = len(chains)
            emitted = 0
            for ci, (xx, ww) in enumerate(chains):
                for kp in range(KP):
                    for half in range(2):
                        nc.tensor.matmul(
                            ps[:, half * 512 : (half + 1) * 512],
                            xx[:, 2 * kp : 2 * kp + 2, :],
                            ww[:, 2 * kp : 2 * kp + 2,
                               col_lo + half * 512 : col_lo + (half + 1) * 512],
                            start=(ci == 0 and kp == 0),
                            stop=(ci == nch - 1 and kp == KP - 1),
                            perf_mode=DR,
                        )
                        emitted += 1
                        if chunk and emitted % chunk == 0:
                            yield

        def qk_pipeline(stage, qi, t):
            """rms norm + rope for q (qi=0) or k (qi=1). stage is the bf16
            SBUF copy of the qkv psum. Returns the t2 tile holding the
            roped+scaled bf16 result (transposed by the caller).

            rope commutes with the rms scale (a per-(p,h) scalar times an
            orthogonal pair-rotation), so the rsqrt multiply is applied
            AFTER the rope add, on the Pool engine."""
            src = stage
            t1 = t1pool.tile([128, 1024], BF16, tag="t1", name="t1")
            sq = t1[:, :]
            nc.vector.tensor_mul(sq, src[:, :], src[:, :])
            sq3 = sq.rearrange("p (h d) -> p h d", d=HD)
            # fold twice before reducing: TensorReduce gets no 2x/4x DVE
            # mode, so shrinking its input is cheaper than reducing wide
            nc.vector.tensor_add(sq3[:, :, 0:32], sq3[:, :, 0:32], sq3[:, :, 32:64])
            nc.vector.tensor_add(sq3[:, :, 0:16], sq3[:, :, 0:16], sq3[:, :, 16:32])
            var = spool.tile([128, H], BF16, tag="var", name="var")
            with nc.allow_low_precision("rms var in bf16 (<0.1% on q)"):
                nc.vector.reduce_sum(
                    var[:, :], sq3[:, :, 0:16], axis=mybir.AxisListType.X
                )
            # rsqrt(var/HD): bf16-bits ln trick + exp-bit-trick + Newton.
            lnv = spool.tile([128, H], F32, tag="lnv", name="lnv")
            nc.vector.tensor_scalar(
                out=lnv[:, :], in0=var[:, :].bitcast(mybir.dt.int16),
                scalar1=-16256 - 128 * 6, scalar2=np.log(2.0) / 128,
                op0=ADD, op1=MULT,
            )
            vv = spool.tile([128, H], F32, tag="vv", name="vv")
            nc.vector.tensor_scalar(
                out=vv[:, :], in0=var[:, :], scalar1=1.0 / HD, scalar2=EPS,
                op0=MULT, op1=ADD,
            )
            r0i = spool.tile([128, H], mybir.dt.int32, tag="r0", name="r0i")
            nc.vector.tensor_scalar(
                out=r0i[:, :], in0=lnv[:, :],
                scalar1=-6051101.6, scalar2=1064866805.0,
                op0=MULT, op1=ADD,
            )
            r0 = r0i[:, :].bitcast(mybir.dt.float32)
            rr = spool.tile([128, H], F32, tag="rr", name="rr")
            e2 = spool.tile([128, H], F32, tag="e2", name="e2")
            cur = r0
            for _ in range(int(os.environ.get("KNEWTON", "2"))):
                nc.vector.tensor_mul(e2[:, :], cur, cur)
                nc.vector.scalar_tensor_tensor(
                    out=e2[:, :], in0=e2[:, :], scalar=-0.5, in1=vv[:, :],
                    op0=MULT, op1=MULT,
                )
                nc.vector.scalar_tensor_tensor(
                    out=rr[:, :], in0=e2[:, :], scalar=1.5, in1=cur,
                    op0=ADD, op1=MULT,
                )
                cur = rr[:, :]

            # rope on the UN-normalized src: t2 = src*C, t1 = swap(src)*S
            ctab = tabs[:, 2 * qi + 0, t, :]
            stab = tabs[:, 2 * qi + 1, t, :]
            qs3 = src[:, :].rearrange("p (h d) -> p h d", d=HD)
            t13 = t1[:, :].rearrange("p (h d) -> p h d", d=HD)
            nc.vector.tensor_mul(
                t13[:, :, 0:32], qs3[:, :, 32:64], _bcast_mid(stab[:, 0:32], H)
            )
            nc.vector.tensor_mul(
                t13[:, :, 32:64], qs3[:, :, 0:32], _bcast_mid(stab[:, 32:64], H)
            )
            t2 = t2pool.tile([128, 1024], BF16, tag="t2", name="t2")
            nc.vector.tensor_mul(
                t2[:, :].rearrange("p (h d) -> p h d", d=HD), qs3,
                _bcast_mid(ctab, H),
            )
            engadd = nc.gpsimd if (KPOOL & 1) else nc.vector
            engadd.tensor_add(t2[:, :], t2[:, :], t1[:, :])
            engadd.tensor_mul(
                t2[:, :].rearrange("p (h d) -> p h d", d=HD),
                t2[:, :].rearrange("p (h d) -> p h d", d=HD),
                _bcast_last(rr[:, :], HD),
            )
            return t2

        def a_step_gen(xt, qkT, v65, t, chunk=CHUNK, qpool=None):
            """one token tile of phase A as a generator: yields between PE
            chunks so the caller can interleave B-phase slots."""
            qpool = qpool or psQ
            qtag = "S" if qpool is psS else "A"
            xhi, xlo = xt
            xts = (xhi[:, :, t * 128 : (t + 1) * 128],
                   xlo[:, :, t * 128 : (t + 1) * 128])
            stages = []
            for qi in range(2):
                ph = qpool.tile([128, 1024], F32, tag=qtag, name="ps_qk")
                yield from qkv_mm(ph[:, :], xts, qi * 1024, chunk=chunk)
                yield  # let the chain drain so the copy joins its queue ready
                stage = qspool.tile([128, 1024], BF16, tag="stage", name="stage")
                if use_bias:
                    nc.vector.scalar_tensor_tensor(
                        out=stage[:, :], in0=ph[:, :], scalar=1.0,
                        in1=bias_qkv[:, qi * 1024 : (qi + 1) * 1024],
                        op0=MULT, op1=ADD,
                    )
                elif os.environ.get("KSTAGE", "act") == "dve":
                    nc.vector.tensor_copy(stage[:, :], ph[:, :])
                else:
                    nc.scalar.copy(stage[:, :], ph[:, :])
                stages.append(stage)
                if qi == 1:
                    tq = qk_pipeline(stages[0], 0, t)
                yield

            # v: 3-chain DR into one [128, 1024] psum; one ACT copy into v65
            psv = qpool.tile([128, 1024], F32, tag=qtag, name="psv")
            yield from qkv_mm(psv[:, :], xts, 2048, chunk=chunk)
            yield
            v3 = v65[:, t, :].rearrange("p (h e) -> p h e", e=HD)
            pv3 = psv[:, :].rearrange("p (h d) -> p h d", d=HD)
            if use_bias:
                nc.vector.scalar_tensor_tensor(
                    out=v3[:, :, :], in0=pv3, scalar=1.0,
                    in1=bias_qkv[:, 2048:3072].rearrange("p (h d) -> p h d", d=HD),
                    op0=MULT, op1=ADD,
                )
            elif os.environ.get("KVCOPY", "dve") == "dve":
                nc.vector.tensor_copy(v3[:, :, :], pv3)
            else:
                nc.scalar.copy(v3[:, :, :], pv3)
            # q-half transpose (its rope chain completed during the k/v mms)
            nc.sync.dma_start_transpose(
                qkT[:, 0:KT, t * 128 : (t + 1) * 128], tq[:, :]
            )
            tk = qk_pipeline(stages[1], 1, t)
            # the transpose WAITS on the rope tail while holding the SP
            # sequencer - delay its emission so it lands nearly-ready
            for _ in range(int(os.environ.get("KTDELAY", "1"))):
                yield
            nc.sync.dma_start_transpose(
                qkT[:, KT : 2 * KT, t * 128 : (t + 1) * 128], tk[:, :]
            )

        def b_phase(attn4, qkT, v65, weave_gen):
            """slot-pipelined phase B: per (h, jt) slot emit S + (delayed)
            exp + (more delayed) O-octet."""
            pending = []  # (h, jt, pt)
            psos = {}
            psd = psDp.tile([128, H, NT], F32, tag="D", name="psd")
            slot = 0

            def emit_o(h, jt, pt):
                if jt == 0:
                    psos[h] = psO2.tile([128, NT, 64], F32, tag="O", name="ps_o")
                ps_o = psos[h]
                vsl = v65[:, jt, h * HD : (h + 1) * HD]
                for ib in range(NT):
                    ptb = pt[:, ib * 128 : (ib + 1) * 128]
                    nc.tensor.matmul(
                        ps_o[:, ib, :], ptb, vsl,
                        start=(jt == 0 and ib == 0),
                        stop=(jt == NT - 1 and ib == NT - 1),
                        skip_group_check=True,
                    )
                    nc.tensor.matmul(
                        psd[:, h, ib : ib + 1], ptb, ones_col[:, :],
                        start=(h == 0 and jt == 0 and ib == 0),
                        stop=(h == H - 1 and jt == NT - 1 and ib == NT - 1),
                        skip_group_check=True,
                    )
                if jt == NT - 1:
                    nrmq.append([h, psos.pop(h), 0])

            def emit_nrm(h, pso):
                # delayed so the reciprocal's wait (the head's last psd
                # matmul) is satisfied before it reaches the DVE queue head
                rec = recpool.tile([128, NT], BF16, tag="rec", name="rec")
                with nc.allow_low_precision("softmax denom recip bf16"):
                    nc.vector.reciprocal(rec[:, :], psd[:, h, :])
                nc.vector.tensor_mul(
                    attn4[:, :, h, :], pso[:, :, :],
                    _bcast_last(rec[:, :], 64),
                )

            def emit_exp(ps_s, dst):
                nc.scalar.activation(
                    dst, ps_s[:, :],
                    mybir.ActivationFunctionType.Exp, scale=0.125,
                )

            front = int(os.environ.get("KFRONT", "64"))
            nrm_delay = int(os.environ.get("KNRMDELAY", "1"))
            expq = []
            nrmq = []
            for h in range(H):
                base = 64 * (h % 2)
                fb = h // 2
                psl = slice(base, base + 64)
                for jt in range(NT):
                    # pop BEFORE emit_o: the next head's O-octet reuses the
                    # single psO buffer, so the normalize read must be
                    # emitted first (write-after-read emission order)
                    for it in nrmq:
                        it[2] += 1
                    if nrmq and nrmq[0][2] >= nrm_delay:
                        hh, pso, _ = nrmq.pop(0)
                        emit_nrm(hh, pso)
                    if weave_gen is not None:
                        next(weave_gen, None)
                        if slot < front:
                            next(weave_gen, None)
                        if slot < int(os.environ.get("KFRONT3", "10")):
                            next(weave_gen, None)
                    ps_s = psS.tile([128, 1024], F32, tag="S", name="ps_s")
                    for ic in range(2):
                        nc.tensor.matmul(
                            ps_s[:, ic * 512 : (ic + 1) * 512],
                            qkT[psl, 8 + fb, jt * 128 : (jt + 1) * 128],
                            qkT[psl, fb, ic * 512 : (ic + 1) * 512],
                            start=True, stop=True,
                            tile_position=(base, 0),
                        )
                    if len(pending) >= O_DELAY:
                        emit_o(*pending.pop(0))
                    # delay exp emission: when it enters the ACT FIFO its S
                    # psum is already complete, so it never blocks the head
                    pt = ptpool.tile([128, 1024], BF16, tag="pt", name="pt")
                    # NOTE: must stay strictly below O_DELAY (the O-octet
                    # consuming pt[n] must be emitted after exp[n] writes it)
                    expq.append((ps_s, pt[:, :]))
                    if len(expq) > min(int(os.environ.get("KEXPDELAY", "1")),
                                       O_DELAY - 1):
                        emit_exp(*expq.pop(0))
                    pending.append((h, jt, pt))
                    slot += 1
            while expq:
                emit_exp(*expq.pop(0))
            for args in pending:
                emit_o(*args)
            for hh, pso, _ in nrmq:
                emit_nrm(hh, pso)

        def c_gen(attn4, b):
            """phase C as a generator (woven into the next batch's B slots):
            attn4 -> XBAR transpose -> attnT -> proj -> DMA out."""

            # attnT transposes have no waits (attn4 is complete): emit them
            # early so they clear the SP queue before the next batch's
            # rope-gated qkT transposes line up behind them
            def transpose(t):
                att = atpool.tile([128, KT, 128], BF16, tag="at", name="attnT")
                nc.sync.dma_start_transpose(att[:, :, :], attn4[:, t, :, :])
                return att

            atts = [transpose(t) for t in range(min(NT, cfg["at"]))]
            yield
            for t in range(NT):
                att = atts[t]
                if t + cfg["at"] < NT:
                    atts.append(transpose(t + cfg["at"]))
                ps_p = psQ.tile([128, 1024], F32,
                                tag=("S" if psQ is psS else "A"), name="ps_p")
                for half in range(2):
                    for k in range(KT):
                        nc.tensor.matmul(
                            ps_p[:, half * 512 : (half + 1) * 512],
                            att[:, k, :],
                            wproj[:, k, half * 512 : (half + 1) * 512],
                            start=(k == 0), stop=(k == KT - 1),
                        )
                    yield
                yield  # let the proj chain finish before the copy queues
                ostage = opool.tile([128, C], F32, tag="ostage", name="ostage")
                if use_bias:
                    nc.vector.tensor_add(
                        ostage[:, :], ps_p[:, :], bias_proj[:, :]
                    )
                elif os.environ.get("KOCOPY", "dve") == "act":
                    nc.scalar.copy(ostage[:, :], ps_p[:, :])
                elif os.environ.get("KOCOPY", "dve") == "dve":
                    nc.vector.tensor_copy(ostage[:, :], ps_p[:, :])
                else:
                    nc.scalar.copy(ostage[:, 0:512], ps_p[:, 0:512])
                    nc.vector.tensor_copy(ostage[:, 512:1024], ps_p[:, 512:1024])
                # out-DMA queue choice: on ACT its wait (the DVE ostage
                # copy) blocks the exp stream; on SP it sits with the
                # transposes (which land nearly-ready now)
                outq = nc.sync if os.environ.get("KOUTQ", "sp") == "sp" else nc.scalar
                outq.dma_start(out=out_d[b, t], in_=ostage[:, :])
                yield

        def alloc_ab():
            qkT = qkpool.tile([128, 2 * KT, N], BF16, tag="qkT", name="qkT")
            v65 = vpool.tile([128, NT, H * HD], BF16, tag="v65", name="v65")
            return qkT, v65

        reps = int(os.environ.get("KREPEAT", "1"))
        batches = [bb for _ in range(reps) for bb in range(BSH)]

        # prologue: weights + A(b0), two token-tile pipelines interleaved
        xt = load_weights(batches[0])
        tiles = alloc_ab()
        from collections import deque

        _done = object()
        gens = [a_step_gen(xt, tiles[0], tiles[1], t,
                           qpool=(psS if t % 2 else psQ)) for t in range(NT)]
        active = deque(gens[:PROLOG])
        gi = PROLOG
        while active:
            g = active.popleft()
            if next(g, _done) is not _done:
                active.append(g)
            elif gi < NT:
                active.append(gens[gi])
                gi += 1

        from itertools import chain as _ichain

        prev_c = None  # (attn4, b) awaiting phase C
        for bi, b in enumerate(batches):
            qkT, v65 = tiles
            attn4 = a4pool.tile([128, NT, H, HD], BF16, tag="attn4", name="attn4")
            wparts = []
            if prev_c is not None:
                wparts.append(c_gen(*prev_c))
            nxt = batches[bi + 1] if bi + 1 < len(batches) else None
            if nxt is not None:
                xt2 = load_x(nxt)
                tiles2 = alloc_ab()

                _ck = int(os.environ.get("KCHUNK0", "3")) if prev_c is None else CHUNK

                def _weave(_xt=xt2, _tl=tiles2, _ck=_ck):
                    for t in range(NT):
                        yield from a_step_gen(_xt, _tl[0], _tl[1], t, chunk=_ck)

                wparts.append(_weave())
            wg = _ichain(*wparts) if wparts else None
            b_phase(attn4, qkT, v65, wg)
            if wg is not None:
                for _ in wg:
                    pass
            prev_c = (attn4, b)
            if nxt is not None:
                xt, tiles = xt2, tiles2
        for _ in c_gen(*prev_c):
            pass

    nc.compile()
    return nc


_NC = {}


def _get_nc(use_bias: bool = False, share_tabs: bool = False):
    key = bool(use_bias)
    if key not in _NC:
        _NC[key] = _build_module(key)
    return _NC[key]


def _rope_tables():
    """cos/sin tables exactly as reference.rope_tables, in float32."""
    grid = int(np.sqrt(N))
    half = HD // 2
    freqs = (1.0 / THETA ** (np.arange(0, half, 2, dtype=np.float32) / half)).astype(
        np.float32
    )
    freqs = np.concatenate([freqs, freqs], axis=0)
    t = np.arange(grid, dtype=np.float32)
    f = np.outer(t, freqs).astype(np.float32)
    fh = np.broadcast_to(f[:, None, :], (grid, grid, half))
    fw = np.broadcast_to(f[None, :, :], (grid, grid, half))
    full = np.concatenate([fh, fw], axis=-1).reshape(-1, HD).astype(np.float32)
    return np.cos(full).astype(np.float32), np.sin(full).astype(np.float32)


def _make_inputs(x, qkv_w, qkv_b, proj_w, proj_b, q_gamma, k_gamma,
                 use_bias=False):
    cos, sin = _rope_tables()
    sgn = np.where(np.arange(HD) < HD // 2, -1.0, 1.0).astype(np.float32)
    swap = (np.arange(HD) + HD // 2) % HD

    def fold(gamma):
        c = (cos * gamma[None, :]).astype(np.float32)
        s = (sin * sgn[None, :] * gamma[swap][None, :]).astype(np.float32)
        return c, s

    cq, sq = fold(q_gamma.astype(np.float32))
    ck, sk = fold(k_gamma.astype(np.float32))
    stack = [cq, sq, ck, sk]
    tabs = np.stack(stack, axis=0).reshape(4, NT, 128, HD).astype(NPBF16)

    ws = (qkv_w.astype(np.float32) * WSCALE).reshape(KT, 128, 3 * C)
    whi = np.ascontiguousarray(ws).astype(NPF8)
    wlo = (ws - whi.astype(np.float32)).astype(NPF8)
    wproj_h = np.ascontiguousarray(
        (proj_w.astype(np.float32) / WSCALE).reshape(KT, 128, C)
    ).astype(NPBF16)

    in_maps = []
    for c in range(N_CORES):
        xc = x[c * BSH : (c + 1) * BSH].astype(np.float32)  # [BSH, N, C]
        xt = np.ascontiguousarray(xc.transpose(0, 2, 1)).reshape(BSH, KT, 128, N)
        xhi = xt.astype(NPF8)
        xlo = (xt - xhi.astype(np.float32)).astype(NPF8)
        m = {
            "xhi": xhi,
            "xlo": xlo,
            "whi": whi,
            "wlo": wlo,
            "wproj": wproj_h,
            "tabs": tabs,
        }
        if use_bias:
            m["bq"] = (qkv_b.astype(np.float32) * WSCALE).astype(NPBF16)
            m["bp"] = proj_b.astype(np.float32).astype(NPBF16)
        in_maps.append(m)
    return in_maps


def _run(in_maps, use_bias=False, trace=False, **kwargs):
    nc = _get_nc(use_bias)
    return run_bass_kernel_spmd(
        nc, in_maps, core_ids=list(range(N_CORES)), trace=trace, **kwargs
    )


def kernel(x, qkv_w, qkv_b, proj_w, proj_b, q_gamma, k_gamma):
    x = np.asarray(x)
    qkv_b = np.asarray(qkv_b)
    proj_b = np.asarray(proj_b)
    use_bias = bool(np.any(qkv_b != 0) or np.any(proj_b != 0))
    q_gamma = np.asarray(q_gamma)
    k_gamma = np.asarray(k_gamma)
    in_maps = _make_inputs(
        x, np.asarray(qkv_w), qkv_b, np.asarray(proj_w), proj_b,
        q_gamma, k_gamma, use_bias=use_bias,
    )
    res = _run(in_maps, use_bias=use_bias)
    outs = [res.results[c]["out"].reshape(BSH, NT * 128, C) for c in range(N_CORES)]
    return np.concatenate(outs, axis=0).astype(np.float32)


# revision 85
# speedup vs baseline: 1.0088x; 1.0088x over previous
"""Trainium2 Bass kernel for nn_Attention_57827439673725.

Dense transformer attention block (B=32, N=1024, C=1024, H=16, hd=64):
  qkv = x @ qkv_w + qkv_b ; q,k rms-normed (per head) and 2D-roped;
  out = softmax(q k^T / sqrt(hd)) v @ proj_w + proj_b

Pure data-parallel over batch across 8 NeuronCores (4 batches each).

v5 = v3 numerics + v4 scheduling. The v4 fp8-attention experiment (fp8
q/k/P/v, DoubleRow S and O) measured 6.6% rel err: with random data the
attention output shrinks as fast as independent noise averages (O is a
random-walk mean), so every fp8 tensor costs its raw ~2.5-4% relative -
each individually over the 2e-2 gate. bf16 attention it is.

Per-core design:
  phase A (per token tile): qkv matmul in fp8e4 DoubleRow mode as a
        3-chain residual product (x_hi+x_lo)(W_hi+W_lo) minus the lo*lo
        term, W pre-scaled by 32 to clear fp8 subnormals (rms-norm makes
        q/k scale-invariant; 1/32 is folded into proj_w for the v path).
        qkv psum -> SBUF bf16 by ONE ACT copy; rms stats on DVE (bf16
        squares, 2 folds + 16-wide reduce since TensorReduce gets no 2x
        mode, bf16-bits ln trick + Newton rsqrt); rope multiplies on DVE;
        the rope ADD and the rsqrt multiply run on the otherwise-idle Pool
        engine (rope commutes with the per-(tok,head) rms scale, so the
        rsqrt multiply fuses after the add). q|k rope bf16 [128, 2048] ->
        one XBAR transpose -> head-major qkT; v lands via one ACT copy.
  phase B (per head h, per j-tile): S^T = k q^T (bf16, K=64, row base
        64*(h%2) via tile_position); ONE ACT table exp over the full
        [128 j, 1024 i] psum -> P^T bf16 (emitted one slot late so it
        never waits at the ACT queue head); O in natural layout: P^T
        block stationary, v65 column-block moving, 64 moving cols per
        128x64 output; parallel 1-col matmuls against a ones column
        accumulate the softmax denominator (single psum group per bank).
        normalize = per-partition DVE reciprocal + one broadcast-last
        multiply into attn4.
  phase C: attn4 -> 8 upfront XBAR transposes (no waits, so they clear
        the SP queue before the rope-gated qkT transposes) -> proj bf16
        -> DVE copy -> DMA out fp32 on the ACT queue (its only wait
        precedes it there).

Slot pipeline: phase B is a slot pipeline; a weave generator (C(b-1)
first, then A(b+1), front-loaded 2-steps/slot early on) yields chunks of
PE work between slots so PE stays busy and p-state-ramped through the
exp-bound stretches. PSUM: shared pool of 3x [128,1024] f32 (S slots,
qkv tiles, proj tiles) + psO [128, NT, 64] + psD [128, H, NT] = 8 banks.
"""

import os
import sys

import numpy as np

for _p in ("/opt/trn_rl_repo",):
    if os.path.isdir(_p) and _p not in sys.path:
        sys.path.insert(0, _p)

import ml_dtypes  # noqa: E402

import concourse.bass as bass  # noqa: E402
import concourse.mybir as mybir  # noqa: E402
import concourse.tile as tile  # noqa: E402
from concourse import bacc  # noqa: E402
from concourse.bass_utils import run_bass_kernel_spmd  # noqa: E402

BF16 = mybir.dt.bfloat16
F32 = mybir.dt.float32
F8 = mybir.dt.float8e4
NPBF16 = ml_dtypes.bfloat16
NPF8 = ml_dtypes.float8_e4m3fn

N_CORES = 8
B, N, C = 32, 1024, 1024
H, HD = 16, 64
BSH = B // N_CORES  # batches per core
NT = N // 128  # token tiles per batch
KT = C // 128  # k tiles over C
KP = KT // 2  # fp8 DoubleRow k-pair count
EPS = 1e-06
THETA = 10000.0
WSCALE = 32.0  # qkv_w prescale (clears fp8e4m3 subnormals)

MULT = mybir.AluOpType.mult
ADD = mybir.AluOpType.add
DR = mybir.MatmulPerfMode.DoubleRow
CHUNK = int(os.environ.get("KCHUNK", "4"))
PROLOG = int(os.environ.get("KPROLOG", "2"))
O_DELAY = int(os.environ.get("KODELAY", "2"))  # in jt slots
KPOOL = int(os.environ.get("KPOOL", "1"))  # 1: rope add+rr-mul on Pool


def _ap_with(ap: bass.AP, dims) -> bass.AP:
    return bass.AP(tensor=ap.tensor, offset=ap.offset, ap=dims)


def _bcast_mid(ap: bass.AP, n: int) -> bass.AP:
    """[P, F] -> [P, n, F] with a 0-step broadcast middle dim."""
    return _ap_with(ap, [ap.ap[0], [0, n], *ap.ap[1:]])


def _bcast_last(ap: bass.AP, n: int) -> bass.AP:
    """[P, F] -> [P, F, n] with a 0-step broadcast last dim."""
    return _ap_with(ap, [*ap.ap, [0, n]])


def _build_module(use_bias: bool):
    nc = bacc.Bacc(
        "TRN2", target_bir_lowering=False, debug=False,
        dynamic_dma_scratch_size=2048,
    )

    xhi_d = nc.dram_tensor("xhi", [BSH, KT, 128, N], F8, kind="ExternalInput")
    xlo_d = nc.dram_tensor("xlo", [BSH, KT, 128, N], F8, kind="ExternalInput")
    whi_d = nc.dram_tensor("whi", [KT, 128, 3 * C], F8, kind="ExternalInput")
    wlo_d = nc.dram_tensor("wlo", [KT, 128, 3 * C], F8, kind="ExternalInput")
    wproj_d = nc.dram_tensor("wproj", [KT, 128, C], BF16, kind="ExternalInput")
    tabs_d = nc.dram_tensor("tabs", [4, NT, 128, HD], BF16, kind="ExternalInput")
    if use_bias:
        bq_d = nc.dram_tensor("bq", [3 * C], BF16, kind="ExternalInput")  # *WSCALE
        bp_d = nc.dram_tensor("bp", [C], BF16, kind="ExternalInput")
    out_d = nc.dram_tensor("out", [BSH, NT, 128, C], F32, kind="ExternalOutput")

    from contextlib import ExitStack

    with ExitStack() as ctx:
        tc = ctx.enter_context(tile.TileContext(nc))
        pool = lambda name, bufs, **kw: ctx.enter_context(  # noqa: E731
            tc.tile_pool(name=name, bufs=bufs, **kw)
        )
        cfg = dict(
            qk=2, v65=2, pt=2, at=2, qs=2, t1=2, t2=2, stats=2,
            rec=2, outs=1, a4=1, psS=3, psQ=0, psO=1,
        )
        for kv in os.environ.get("KBUFS", "").split(","):
            if kv:
                kk, vv = kv.split("=")
                cfg[kk] = int(vv)

        wpool = pool("weights", 1)
        cpool = pool("consts", 1)
        xtpool = pool("xt", 1)
        qkpool = pool("qkT", cfg["qk"])
        vpool = pool("v65", cfg["v65"])
        ptpool = pool("pt", cfg["pt"])
        a4pool = pool("attn4", cfg["a4"])
        atpool = pool("attnT", cfg["at"])
        qspool = pool("qs", cfg["qs"])
        t1pool = pool("t1", cfg["t1"])
        t2pool = pool("t2", cfg["t2"])
        spool = pool("stats", cfg["stats"])
        recpool = pool("rec", cfg["rec"])
        opool = pool("outs", cfg["outs"])
        psS = pool("psS", cfg["psS"], space="PSUM")
        psQ = pool("psQ", cfg["psQ"], space="PSUM") if cfg["psQ"] else psS
        psO2 = pool("psO", cfg["psO"], space="PSUM")
        psDp = pool("psD", 1, space="PSUM")

        # ---- persistent weights / constants ----
        whi = wpool.tile([128, KT, 3 * C], F8, tag="whi")
        wlo = wpool.tile([128, KT, 3 * C], F8, tag="wlo")
        wproj = wpool.tile([128, KT, C], BF16, tag="wproj")

        tabs = cpool.tile([128, 4, NT, HD], BF16, tag="tabs")
        for i in range(4):
            nc.sync.dma_start(
                out=tabs[:, i, :, :], in_=tabs_d[i].rearrange("t p d -> p t d")
            )
        if use_bias:
            bias_qkv = cpool.tile([128, 3 * C], BF16, tag="bq")
            bq_ap = bq_d[:]
            nc.sync.dma_start(
                out=bias_qkv[:, :], in_=_ap_with(bq_ap, [[0, 128], *bq_ap.ap])
            )
            bias_proj = cpool.tile([128, C], BF16, tag="bp")
            bp_ap = bp_d[:]
            nc.sync.dma_start(
                out=bias_proj[:, :], in_=_ap_with(bp_ap, [[0, 128], *bp_ap.ap])
            )
        ones_col = cpool.tile([128, 1], BF16, tag="ones1")
        nc.vector.memset(ones_col[:, :], 1.0)

        def load_x(b):
            xhi = xtpool.tile([128, KT, N], F8, tag="xhi", name="xhi")
            nc.sync.dma_start(
                out=xhi[:, :, :], in_=xhi_d[b].rearrange("k p n -> p k n")
            )
            xlo = xtpool.tile([128, KT, N], F8, tag="xlo", name="xlo")
            nc.sync.dma_start(
                out=xlo[:, :, :], in_=xlo_d[b].rearrange("k p n -> p k n")
            )
            return xhi, xlo

        def load_weights(b0):
            # first-consumed slices first: the opening q-chain needs only
            # whi/wlo cols 0:512 and x tile 0 - land those in small DMAs so
            # PE starts early, then stream the rest
            # per-k-pair slices: the opening q-chain starts after ~128KB
            # instead of waiting for the full 1.5MB column block
            nc.sync.dma_start(
                out=whi[:, 0:2, 0:512],
                in_=whi_d[0:2, :, 0:512].rearrange("k p n -> p k n"),
            )
            xhi = xtpool.tile([128, KT, N], F8, tag="xhi", name="xhi")
            nc.sync.dma_start(
                out=xhi[:, :, 0:128], in_=xhi_d[b0, :, :, 0:128].rearrange("k p n -> p k n")
            )
            nc.sync.dma_start(
                out=whi[:, 2:, 0:512],
                in_=whi_d[2:, :, 0:512].rearrange("k p n -> p k n"),
            )
            nc.sync.dma_start(
                out=wlo[:, 0:2, 0:512],
                in_=wlo_d[0:2, :, 0:512].rearrange("k p n -> p k n"),
            )
            nc.sync.dma_start(
                out=wlo[:, 2:, 0:512],
                in_=wlo_d[2:, :, 0:512].rearrange("k p n -> p k n"),
            )
            xlo = xtpool.tile([128, KT, N], F8, tag="xlo", name="xlo")
            nc.sync.dma_start(
                out=xlo[:, :, 0:128], in_=xlo_d[b0, :, :, 0:128].rearrange("k p n -> p k n")
            )
            nc.sync.dma_start(
                out=whi[:, :, 512:1024],
                in_=whi_d[:, :, 512:1024].rearrange("k p n -> p k n"),
            )
            nc.sync.dma_start(
                out=wlo[:, :, 512:1024],
                in_=wlo_d[:, :, 512:1024].rearrange("k p n -> p k n"),
            )
            nc.sync.dma_start(
                out=xhi[:, :, 128:], in_=xhi_d[b0, :, :, 128:].rearrange("k p n -> p k n")
            )
            nc.sync.dma_start(
                out=xlo[:, :, 128:], in_=xlo_d[b0, :, :, 128:].rearrange("k p n -> p k n")
            )
            nc.sync.dma_start(
                out=whi[:, :, 1024:],
                in_=whi_d[:, :, 1024:].rearrange("k p n -> p k n"),
            )
            nc.sync.dma_start(
                out=wlo[:, :, 1024:],
                in_=wlo_d[:, :, 1024:].rearrange("k p n -> p k n"),
            )
            nc.sync.dma_start(
                out=wproj[:, :, :], in_=wproj_d[:, :, :].rearrange("k p n -> p k n")
            )
            return xhi, xlo

        def qkv_mm(ps, xts, col_lo, chunk=None):
            """3-chain fp8 DR product into ps[:, 0:1024].

            Generator when chunk is set: yields between groups of `chunk`
            matmuls so the caller can interleave B-phase slots.
            """
            xhi, xlo = xts
            chains = [(xhi, whi), (xhi, wlo), (xlo, whi)]
            nch = len(chains)
            emitted = 0
            for ci, (xx, ww) in enumerate(chains):
                for kp in range(KP):
                    for half in range(2):
                        nc.tensor.matmul(
                            ps[:, half * 512 : (half + 1) * 512],
                            xx[:, 2 * kp : 2 * kp + 2, :],
                            ww[:, 2 * kp : 2 * kp + 2,
                               col_lo + half * 512 : col_lo + (half + 1) * 512],
                            start=(ci == 0 and kp == 0),
                            stop=(ci == nch - 1 and kp == KP - 1),
                            perf_mode=DR,
                        )
                        emitted += 1
                        if chunk and emitted % chunk == 0:
                            yield

        def qk_pipeline(stage, qi, t):
            """rms norm + rope for q (qi=0) or k (qi=1). stage is the bf16
            SBUF copy of the qkv psum. Returns the t2 tile holding the
            roped+scaled bf16 result (transposed by the caller).

            rope commutes with the rms scale (a per-(p,h) scalar times an
            orthogonal pair-rotation), so the rsqrt multiply is applied
            AFTER the rope add, on the Pool engine."""
            src = stage
            t1 = t1pool.tile([128, 1024], BF16, tag="t1", name="t1")
            sq = t1[:, :]
            nc.vector.tensor_mul(sq, src[:, :], src[:, :])
            sq3 = sq.rearrange("p (h d) -> p h d", d=HD)
            # fold twice before reducing: TensorReduce gets no 2x/4x DVE
            # mode, so shrinking its input is cheaper than reducing wide
            nc.vector.tensor_add(sq3[:, :, 0:32], sq3[:, :, 0:32], sq3[:, :, 32:64])
            nc.vector.tensor_add(sq3[:, :, 0:16], sq3[:, :, 0:16], sq3[:, :, 16:32])
            var = spool.tile([128, H], BF16, tag="var", name="var")
            with nc.allow_low_precision("rms var in bf16 (<0.1% on q)"):
                nc.vector.reduce_sum(
                    var[:, :], sq3[:, :, 0:16], axis=mybir.AxisListType.X
                )
            # rsqrt(var/HD): bf16-bits ln trick + exp-bit-trick + Newton.
            lnv = spool.tile([128, H], F32, tag="lnv", name="lnv")
            nc.vector.tensor_scalar(
                out=lnv[:, :], in0=var[:, :].bitcast(mybir.dt.int16),
                scalar1=-16256 - 128 * 6, scalar2=np.log(2.0) / 128,
                op0=ADD, op1=MULT,
            )
            vv = spool.tile([128, H], F32, tag="vv", name="vv")
            nc.vector.tensor_scalar(
                out=vv[:, :], in0=var[:, :], scalar1=1.0 / HD, scalar2=EPS,
                op0=MULT, op1=ADD,
            )
            r0i = spool.tile([128, H], mybir.dt.int32, tag="r0", name="r0i")
            nc.vector.tensor_scalar(
                out=r0i[:, :], in0=lnv[:, :],
                scalar1=-6051101.6, scalar2=1064866805.0,
                op0=MULT, op1=ADD,
            )
            r0 = r0i[:, :].bitcast(mybir.dt.float32)
            rr = spool.tile([128, H], F32, tag="rr", name="rr")
            e2 = spool.tile([128, H], F32, tag="e2", name="e2")
            cur = r0
            for _ in range(int(os.environ.get("KNEWTON", "2"))):
                nc.vector.tensor_mul(e2[:, :], cur, cur)
                nc.vector.scalar_tensor_tensor(
                    out=e2[:, :], in0=e2[:, :], scalar=-0.5, in1=vv[:, :],
                    op0=MULT, op1=MULT,
                )
                nc.vector.scalar_tensor_tensor(
                    out=rr[:, :], in0=e2[:, :], scalar=1.5, in1=cur,
                    op0=ADD, op1=MULT,
                )
                cur = rr[:, :]

            # rope on the UN-normalized src: t2 = src*C, t1 = swap(src)*S
            ctab = tabs[:, 2 * qi + 0, t, :]
            stab = tabs[:, 2 * qi + 1, t, :]
            qs3 = src[:, :].rearrange("p (h d) -> p h d", d=HD)
            t13 = t1[:, :].rearrange("p (h d) -> p h d", d=HD)
            nc.vector.tensor_mul(
                t13[:, :, 0:32], qs3[:, :, 32:64], _bcast_mid(stab[:, 0:32], H)
            )
            nc.vector.tensor_mul(
                t13[:, :, 32:64], qs3[:, :, 0:32], _bcast_mid(stab[:, 32:64], H)
            )
            t2 = t2pool.tile([128, 1024], BF16, tag="t2", name="t2")
            nc.vector.tensor_mul(
                t2[:, :].rearrange("p (h d) -> p h d", d=HD), qs3,
                _bcast_mid(ctab, H),
            )
            engadd = nc.gpsimd if (KPOOL & 1) else nc.vector
            engadd.tensor_add(t2[:, :], t2[:, :], t1[:, :])
            engadd.tensor_mul(
                t2[:, :].rearrange("p (h d) -> p h d", d=HD),
                t2[:, :].rearrange("p (h d) -> p h d", d=HD),
                _bcast_last(rr[:, :], HD),
            )
            return t2

        def a_step_gen(xt, qkT, v65, t, chunk=CHUNK, qpool=None):
            """one token tile of phase A as a generator: yields between PE
            chunks so the caller can interleave B-phase slots."""
            qpool = qpool or psQ
            qtag = "S" if qpool is psS else "A"
            xhi, xlo = xt
            xts = (xhi[:, :, t * 128 : (t + 1) * 128],
                   xlo[:, :, t * 128 : (t + 1) * 128])
            stages = []
            for qi in range(2):
                ph = qpool.tile([128, 1024], F32, tag=qtag, name="ps_qk")
                yield from qkv_mm(ph[:, :], xts, qi * 1024, chunk=chunk)
                yield  # let the chain drain so the copy joins its queue ready
                stage = qspool.tile([128, 1024], BF16, tag="stage", name="stage")
                if use_bias:
                    nc.vector.scalar_tensor_tensor(
                        out=stage[:, :], in0=ph[:, :], scalar=1.0,
                        in1=bias_qkv[:, qi * 1024 : (qi + 1) * 1024],
                        op0=MULT, op1=ADD,
                    )
                elif os.environ.get("KSTAGE", "act") == "dve":
                    nc.vector.tensor_copy(stage[:, :], ph[:, :])
                else:
                    nc.scalar.copy(stage[:, :], ph[:, :])
                stages.append(stage)
                if qi == 1:
                    tq = qk_pipeline(stages[0], 0, t)
                yield

            # v: 3-chain DR into one [128, 1024] psum; one ACT copy into v65
            psv = qpool.tile([128, 1024], F32, tag=qtag, name="psv")
            yield from qkv_mm(psv[:, :], xts, 2048, chunk=chunk)
            yield
            v3 = v65[:, t, :].rearrange("p (h e) -> p h e", e=HD)
            pv3 = psv[:, :].rearrange("p (h d) -> p h d", d=HD)
            if use_bias:
                nc.vector.scalar_tensor_tensor(
                    out=v3[:, :, :], in0=pv3, scalar=1.0,
                    in1=bias_qkv[:, 2048:3072].rearrange("p (h d) -> p h d", d=HD),
                    op0=MULT, op1=ADD,
                )
            elif os.environ.get("KVCOPY", "dve") == "dve":
                nc.vector.tensor_copy(v3[:, :, :], pv3)
            else:
                nc.scalar.copy(v3[:, :, :], pv3)
            # q-half transpose (its rope chain completed during the k/v mms)
            nc.sync.dma_start_transpose(
                qkT[:, 0:KT, t * 128 : (t + 1) * 128], tq[:, :]
            )
            tk = qk_pipeline(stages[1], 1, t)
            # the transpose WAITS on the rope tail while holding the SP
            # sequencer - delay its emission so it lands nearly-ready
            for _ in range(int(os.environ.get("KTDELAY", "1"))):
                yield
            nc.sync.dma_start_transpose(
                qkT[:, KT : 2 * KT, t * 128 : (t + 1) * 128], tk[:, :]
            )

        def b_phase(attn4, qkT, v65, weave_gen):
            """slot-pipelined phase B: per (h, jt) slot emit S + (delayed)
            exp + (more delayed) O-octet."""
            pending = []  # (h, jt, pt)
            psos = {}
            psd = psDp.tile([128, H, NT], F32, tag="D", name="psd")
            slot = 0

            def emit_o(h, jt, pt):
                if jt == 0:
                    psos[h] = psO2.tile([128, NT, 64], F32, tag="O", name="ps_o")
                ps_o = psos[h]
                vsl = v65[:, jt, h * HD : (h + 1) * HD]
                for ib in range(NT):
                    ptb = pt[:, ib * 128 : (ib + 1) * 128]
                    nc.tensor.matmul(
                        ps_o[:, ib, :], ptb, vsl,
                        start=(jt == 0 and ib == 0),
                        stop=(jt == NT - 1 and ib == NT - 1),
                        skip_group_check=True,
                    )
                    nc.tensor.matmul(
                        psd[:, h, ib : ib + 1], ptb, ones_col[:, :],
                        start=(h == 0 and jt == 0 and ib == 0),
                        stop=(h == H - 1 and jt == NT - 1 and ib == NT - 1),
                        skip_group_check=True,
                    )
                if jt == NT - 1:
                    nrmq.append([h, psos.pop(h), 0])

            def emit_nrm(h, pso):
                # delayed so the reciprocal's wait (the head's last psd
                # matmul) is satisfied before it reaches the DVE queue head
                rec = recpool.tile([128, NT], BF16, tag="rec", name="rec")
                with nc.allow_low_precision("softmax denom recip bf16"):
                    nc.vector.reciprocal(rec[:, :], psd[:, h, :])
                nc.vector.tensor_mul(
                    attn4[:, :, h, :], pso[:, :, :],
                    _bcast_last(rec[:, :], 64),
                )

            def emit_exp(ps_s, dst):
                nc.scalar.activation(
                    dst, ps_s[:, :],
                    mybir.ActivationFunctionType.Exp, scale=0.125,
                )

            front = int(os.environ.get("KFRONT", "64"))
            nrm_delay = int(os.environ.get("KNRMDELAY", "1"))
            expq = []
            nrmq = []
            for h in range(H):
                base = 64 * (h % 2)
                fb = h // 2
                psl = slice(base, base + 64)
                for jt in range(NT):
                    # pop BEFORE emit_o: the next head's O-octet reuses the
                    # single psO buffer, so the normalize read must be
                    # emitted first (write-after-read emission order)
                    for it in nrmq:
                        it[2] += 1
                    if nrmq and nrmq[0][2] >= nrm_delay:
                        hh, pso, _ = nrmq.pop(0)
                        emit_nrm(hh, pso)
                    if weave_gen is not None:
                        next(weave_gen, None)
                        if slot < front:
                            next(weave_gen, None)
                        if slot < int(os.environ.get("KFRONT3", "10")):
                            next(weave_gen, None)
                    ps_s = psS.tile([128, 1024], F32, tag="S", name="ps_s")
                    for ic in range(2):
                        nc.tensor.matmul(
                            ps_s[:, ic * 512 : (ic + 1) * 512],
                            qkT[psl, 8 + fb, jt * 128 : (jt + 1) * 128],
                            qkT[psl, fb, ic * 512 : (ic + 1) * 512],
                            start=True, stop=True,
                            tile_position=(base, 0),
                        )
                    if len(pending) >= O_DELAY:
                        emit_o(*pending.pop(0))
                    # delay exp emission: when it enters the ACT FIFO its S
                    # psum is already complete, so it never blocks the head
                    pt = ptpool.tile([128, 1024], BF16, tag="pt", name="pt")
                    # NOTE: must stay strictly below O_DELAY (the O-octet
                    # consuming pt[n] must be emitted after exp[n] writes it)
                    expq.append((ps_s, pt[:, :]))
                    if len(expq) > min(int(os.environ.get("KEXPDELAY", "1")),
                                       O_DELAY - 1):
                        emit_exp(*expq.pop(0))
                    pending.append((h, jt, pt))
                    slot += 1
            while expq:
                emit_exp(*expq.pop(0))
            for args in pending:
                emit_o(*args)
            for hh, pso, _ in nrmq:
                emit_nrm(hh, pso)

        def c_gen(attn4, b):
            """phase C as a generator (woven into the next batch's B slots):
            attn4 -> XBAR transpose -> attnT -> proj -> DMA out."""

            # attnT transposes have no waits (attn4 is complete): emit them
            # early so they clear the SP queue before the next batch's
            # rope-gated qkT transposes line up behind them
            def transpose(t):
                att = atpool.tile([128, KT, 128], BF16, tag="at", name="attnT")
                nc.sync.dma_start_transpose(att[:, :, :], attn4[:, t, :, :])
                return att

            atts = [transpose(t) for t in range(min(NT, cfg["at"]))]
            yield
            for t in range(NT):
                att = atts[t]
                if t + cfg["at"] < NT:
                    atts.append(transpose(t + cfg["at"]))
                ps_p = psQ.tile([128, 1024], F32,
                                tag=("S" if psQ is psS else "A"), name="ps_p")
                for half in range(2):
                    for k in range(KT):
                        nc.tensor.matmul(
                            ps_p[:, half * 512 : (half + 1) * 512],
                            att[:, k, :],
                            wproj[:, k, half * 512 : (half + 1) * 512],
                            start=(k == 0), stop=(k == KT - 1),
                        )
                    yield
                yield  # let the proj chain finish before the copy queues
                ostage = opool.tile([128, C], F32, tag="ostage", name="ostage")
                if use_bias:
                    nc.vector.tensor_add(
                        ostage[:, :], ps_p[:, :], bias_proj[:, :]
                    )
                elif os.environ.get("KOCOPY", "dve") == "act":
                    nc.scalar.copy(ostage[:, :], ps_p[:, :])
                elif os.environ.get("KOCOPY", "dve") == "dve":
                    nc.vector.tensor_copy(ostage[:, :], ps_p[:, :])
                else:
                    nc.scalar.copy(ostage[:, 0:512], ps_p[:, 0:512])
                    nc.vector.tensor_copy(ostage[:, 512:1024], ps_p[:, 512:1024])
                # out-DMA queue choice: on ACT its wait (the DVE ostage
                # copy) blocks the exp stream; on SP it sits with the
                # transposes (which land nearly-ready now)
                outq = nc.sync if os.environ.get("KOUTQ", "sp") == "sp" else nc.scalar
                outq.dma_start(out=out_d[b, t], in_=ostage[:, :])
                yield

        def alloc_ab():
            qkT = qkpool.tile([128, 2 * KT, N], BF16, tag="qkT", name="qkT")
            v65 = vpool.tile([128, NT, H * HD], BF16, tag="v65", name="v65")
            return qkT, v65

        reps = int(os.environ.get("KREPEAT", "1"))
        batches = [bb for _ in range(reps) for bb in range(BSH)]

        # prologue: weights + A(b0), two token-tile pipelines interleaved
        xt = load_weights(batches[0])
        tiles = alloc_ab()
        from collections import deque

        _done = object()
        gens = [a_step_gen(xt, tiles[0], tiles[1], t,
                           qpool=(psS if t % 2 else psQ)) for t in range(NT)]
        active = deque(gens[:PROLOG])
        gi = PROLOG
        while active:
            g = active.popleft()
            if next(g, _done) is not _done:
                active.append(g)
            elif gi < NT:
                active.append(gens[gi])
                gi += 1

        from itertools import chain as _ichain

        prev_c = None  # (attn4, b) awaiting phase C
        for bi, b in enumerate(batches):
            qkT, v65 = tiles
            attn4 = a4pool.tile([128, NT, H, HD], BF16, tag="attn4", name="attn4")
            wparts = []
            if prev_c is not None:
                wparts.append(c_gen(*prev_c))
            nxt = batches[bi + 1] if bi + 1 < len(batches) else None
            if nxt is not None:
                xt2 = load_x(nxt)
                tiles2 = alloc_ab()

                _ck = int(os.environ.get("KCHUNK0", "3")) if prev_c is None else CHUNK

                def _weave(_xt=xt2, _tl=tiles2, _ck=_ck):
                    for t in range(NT):
                        yield from a_step_gen(_xt, _tl[0], _tl[1], t, chunk=_ck)

                wparts.append(_weave())
            wg = _ichain(*wparts) if wparts else None
            b_phase(attn4, qkT, v65, wg)
            if wg is not None:
                for _ in wg:
                    pass
            prev_c = (attn4, b)
            if nxt is not None:
                xt, tiles = xt2, tiles2
        for _ in c_gen(*prev_c):
            pass

    nc.compile()
    return nc


_NC = {}


def _get_nc(use_bias: bool = False, share_tabs: bool = False):
    key = bool(use_bias)
    if key not in _NC:
        _NC[key] = _build_module(key)
    return _NC[key]


def _rope_tables():
    """cos/sin tables exactly as reference.rope_tables, in float32."""
    grid = int(np.sqrt(N))
    half = HD // 2
    freqs = (1.0 / THETA ** (np.arange(0, half, 2, dtype=np.float32) / half)).astype(
        np.float32
    )
    freqs = np.concatenate([freqs, freqs], axis=0)
    t = np.arange(grid, dtype=np.float32)
    f = np.outer(t, freqs).astype(np.float32)
    fh = np.broadcast_to(f[:, None, :], (grid, grid, half))
    fw = np.broadcast_to(f[None, :, :], (grid, grid, half))
    full = np.concatenate([fh, fw], axis=-1).reshape(-1, HD).astype(np.float32)
    return np.cos(full).astype(np.float32), np.sin(full).astype(np.float32)


def _make_inputs(x, qkv_w, qkv_b, proj_w, proj_b, q_gamma, k_gamma,
                 use_bias=False):
    cos, sin = _rope_tables()
    sgn = np.where(np.arange(HD) < HD // 2, -1.0, 1.0).astype(np.float32)
    swap = (np.arange(HD) + HD // 2) % HD

    def fold(gamma):
        c = (cos * gamma[None, :]).astype(np.float32)
        s = (sin * sgn[None, :] * gamma[swap][None, :]).astype(np.float32)
        return c, s

    cq, sq = fold(q_gamma.astype(np.float32))
    ck, sk = fold(k_gamma.astype(np.float32))
    stack = [cq, sq, ck, sk]
    tabs = np.stack(stack, axis=0).reshape(4, NT, 128, HD).astype(NPBF16)

    ws = (qkv_w.astype(np.float32) * WSCALE).reshape(KT, 128, 3 * C)
    whi = np.ascontiguousarray(ws).astype(NPF8)
    wlo = (ws - whi.astype(np.float32)).astype(NPF8)
    wproj_h = np.ascontiguousarray(
        (proj_w.astype(np.float32) / WSCALE).reshape(KT, 128, C)
    ).astype(NPBF16)

    in_maps = []
    for c in range(N_CORES):
        xc = x[c * BSH : (c + 1) * BSH].astype(np.float32)  # [BSH, N, C]
        xt = np.ascontiguousarray(xc.transpose(0, 2, 1)).reshape(BSH, KT, 128, N)
        xhi = xt.astype(NPF8)
        xlo = (xt - xhi.astype(np.float32)).astype(NPF8)
        m = {
            "xhi": xhi,
            "xlo": xlo,
            "whi": whi,
            "wlo": wlo,
            "wproj": wproj_h,
            "tabs": tabs,
        }
        if use_bias:
            m["bq"] = (qkv_b.astype(np.float32) * WSCALE).astype(NPBF16)
            m["bp"] = proj_b.astype(np.float32).astype(NPBF16)
        in_maps.append(m)
    return in_maps


def _run(in_maps, use_bias=False, trace=False, **kwargs):
    nc = _get_nc(use_bias)
    return run_bass_kernel_spmd(
        nc, in_maps, core_ids=list(range(N_CORES)), trace=trace, **kwargs
    )


def kernel(x, qkv_w, qkv_b, proj_w, proj_b, q_gamma, k_gamma):
    x = np.asarray(x)
    qkv_b = np.asarray(qkv_b)
    proj_b = np.asarray(proj_b)
    use_bias = bool(np.any(qkv_b != 0) or np.any(proj_b != 0))
    q_gamma = np.asarray(q_gamma)
    k_gamma = np.asarray(k_gamma)
    in_maps = _make_inputs(
        x, np.asarray(qkv_w), qkv_b, np.asarray(proj_w), proj_b,
        q_gamma, k_gamma, use_bias=use_bias,
    )
    res = _run(in_maps, use_bias=use_bias)
    outs = [res.results[c]["out"].reshape(BSH, NT * 128, C) for c in range(N_CORES)]
    return np.concatenate(outs, axis=0).astype(np.float32)


# revision 86
# speedup vs baseline: 1.0372x; 1.0282x over previous
"""Trainium2 Bass kernel for nn_Attention_57827439673725.

Dense transformer attention block (B=32, N=1024, C=1024, H=16, hd=64):
  qkv = x @ qkv_w + qkv_b ; q,k rms-normed (per head) and 2D-roped;
  out = softmax(q k^T / sqrt(hd)) v @ proj_w + proj_b

Pure data-parallel over batch across 8 NeuronCores (4 batches each).

v5 = v3 numerics + v4 scheduling. The v4 fp8-attention experiment (fp8
q/k/P/v, DoubleRow S and O) measured 6.6% rel err: with random data the
attention output shrinks as fast as independent noise averages (O is a
random-walk mean), so every fp8 tensor costs its raw ~2.5-4% relative -
each individually over the 2e-2 gate. bf16 attention it is.

Per-core design:
  phase A (per token tile): qkv matmul in fp8e4 DoubleRow mode as a
        3-chain residual product (x_hi+x_lo)(W_hi+W_lo) minus the lo*lo
        term, W pre-scaled by 32 to clear fp8 subnormals (rms-norm makes
        q/k scale-invariant; 1/32 is folded into proj_w for the v path).
        qkv psum -> SBUF bf16 by ONE ACT copy; rms stats on DVE (bf16
        squares, 2 folds + 16-wide reduce since TensorReduce gets no 2x
        mode, bf16-bits ln trick + Newton rsqrt); rope multiplies on DVE;
        the rope ADD and the rsqrt multiply run on the otherwise-idle Pool
        engine (rope commutes with the per-(tok,head) rms scale, so the
        rsqrt multiply fuses after the add). q|k rope bf16 [128, 2048] ->
        one XBAR transpose -> head-major qkT; v lands via one ACT copy.
  phase B (per head h, per j-tile): S^T = k q^T (bf16, K=64, row base
        64*(h%2) via tile_position); ONE ACT table exp over the full
        [128 j, 1024 i] psum -> P^T bf16 (emitted one slot late so it
        never waits at the ACT queue head); O in natural layout: P^T
        block stationary, v65 column-block moving, 64 moving cols per
        128x64 output; parallel 1-col matmuls against a ones column
        accumulate the softmax denominator (single psum group per bank).
        normalize = per-partition DVE reciprocal + one broadcast-last
        multiply into attn4.
  phase C: attn4 -> 8 upfront XBAR transposes (no waits, so they clear
        the SP queue before the rope-gated qkT transposes) -> proj bf16
        -> DVE copy -> DMA out fp32 on the ACT queue (its only wait
        precedes it there).

Slot pipeline: phase B is a slot pipeline; a weave generator (C(b-1)
first, then A(b+1), front-loaded 2-steps/slot early on) yields chunks of
PE work between slots so PE stays busy and p-state-ramped through the
exp-bound stretches. PSUM: shared pool of 3x [128,1024] f32 (S slots,
qkv tiles, proj tiles) + psO [128, NT, 64] + psD [128, H, NT] = 8 banks.
"""

import os
import sys

import numpy as np

for _p in ("/opt/trn_rl_repo",):
    if os.path.isdir(_p) and _p not in sys.path:
        sys.path.insert(0, _p)

import ml_dtypes  # noqa: E402

import concourse.bass as bass  # noqa: E402
import concourse.mybir as mybir  # noqa: E402
import concourse.tile as tile  # noqa: E402
from concourse import bacc  # noqa: E402
from concourse.bass_utils import run_bass_kernel_spmd  # noqa: E402

BF16 = mybir.dt.bfloat16
F32 = mybir.dt.float32
F8 = mybir.dt.float8e4
NPBF16 = ml_dtypes.bfloat16
NPF8 = ml_dtypes.float8_e4m3fn

N_CORES = 8
B, N, C = 32, 1024, 1024
H, HD = 16, 64
BSH = B // N_CORES  # batches per core
NT = N // 128  # token tiles per batch
KT = C // 128  # k tiles over C
KP = KT // 2  # fp8 DoubleRow k-pair count
EPS = 1e-06
THETA = 10000.0
WSCALE = 32.0  # qkv_w prescale (clears fp8e4m3 subnormals)

MULT = mybir.AluOpType.mult
ADD = mybir.AluOpType.add
DR = mybir.MatmulPerfMode.DoubleRow
CHUNK = int(os.environ.get("KCHUNK", "4"))
PROLOG = int(os.environ.get("KPROLOG", "2"))
O_DELAY = int(os.environ.get("KODELAY", "2"))  # in jt slots
KPOOL = int(os.environ.get("KPOOL", "1"))  # 1: rope add+rr-mul on Pool


def _ap_with(ap: bass.AP, dims) -> bass.AP:
    return bass.AP(tensor=ap.tensor, offset=ap.offset, ap=dims)


def _bcast_mid(ap: bass.AP, n: int) -> bass.AP:
    """[P, F] -> [P, n, F] with a 0-step broadcast middle dim."""
    return _ap_with(ap, [ap.ap[0], [0, n], *ap.ap[1:]])


def _bcast_last(ap: bass.AP, n: int) -> bass.AP:
    """[P, F] -> [P, F, n] with a 0-step broadcast last dim."""
    return _ap_with(ap, [*ap.ap, [0, n]])


def _build_module(use_bias: bool, share_tabs: bool = False):
    nc = bacc.Bacc(
        "TRN2", target_bir_lowering=False, debug=False,
        dynamic_dma_scratch_size=2048,
    )

    xhi_d = nc.dram_tensor("xhi", [BSH, KT, 128, N], F8, kind="ExternalInput")
    xlo_d = nc.dram_tensor("xlo", [BSH, KT, 128, N], F8, kind="ExternalInput")
    whi_d = nc.dram_tensor("whi", [KT, 128, 3 * C], F8, kind="ExternalInput")
    wlo_d = nc.dram_tensor("wlo", [KT, 128, 3 * C], F8, kind="ExternalInput")
    wproj_d = nc.dram_tensor("wproj", [KT, 128, C], BF16, kind="ExternalInput")
    n_tab = 2 if share_tabs else 4
    tabs_d = nc.dram_tensor("tabs", [n_tab, NT, 128, HD], BF16, kind="ExternalInput")
    if use_bias:
        bq_d = nc.dram_tensor("bq", [3 * C], BF16, kind="ExternalInput")  # *WSCALE
        bp_d = nc.dram_tensor("bp", [C], BF16, kind="ExternalInput")
    out_d = nc.dram_tensor("out", [BSH, NT, 128, C], F32, kind="ExternalOutput")

    from contextlib import ExitStack

    with ExitStack() as ctx:
        tc = ctx.enter_context(tile.TileContext(nc))
        pool = lambda name, bufs, **kw: ctx.enter_context(  # noqa: E731
            tc.tile_pool(name=name, bufs=bufs, **kw)
        )
        cfg = dict(
            qk=2, v65=2, pt=(3 if share_tabs else 2), at=2, qs=2, t1=2,
            t2=2, stats=2, rec=2, outs=1, a4=1, psS=3, psQ=0, psO=1,
        )
        for kv in os.environ.get("KBUFS", "").split(","):
            if kv:
                kk, vv = kv.split("=")
                cfg[kk] = int(vv)

        wpool = pool("weights", 1)
        cpool = pool("consts", 1)
        xtpool = pool("xt", 1)
        qkpool = pool("qkT", cfg["qk"])
        vpool = pool("v65", cfg["v65"])
        ptpool = pool("pt", cfg["pt"])
        a4pool = pool("attn4", cfg["a4"])
        atpool = pool("attnT", cfg["at"])
        qspool = pool("qs", cfg["qs"])
        t1pool = pool("t1", cfg["t1"])
        t2pool = pool("t2", cfg["t2"])
        spool = pool("stats", cfg["stats"])
        recpool = pool("rec", cfg["rec"])
        opool = pool("outs", cfg["outs"])
        psS = pool("psS", cfg["psS"], space="PSUM")
        psQ = pool("psQ", cfg["psQ"], space="PSUM") if cfg["psQ"] else psS
        psO2 = pool("psO", cfg["psO"], space="PSUM")
        psDp = pool("psD", 1, space="PSUM")

        # ---- persistent weights / constants ----
        whi = wpool.tile([128, KT, 3 * C], F8, tag="whi")
        wlo = wpool.tile([128, KT, 3 * C], F8, tag="wlo")
        wproj = wpool.tile([128, KT, C], BF16, tag="wproj")

        tabs = cpool.tile([128, n_tab, NT, HD], BF16, tag="tabs")
        for i in range(n_tab):
            nc.sync.dma_start(
                out=tabs[:, i, :, :], in_=tabs_d[i].rearrange("t p d -> p t d")
            )
        if use_bias:
            bias_qkv = cpool.tile([128, 3 * C], BF16, tag="bq")
            bq_ap = bq_d[:]
            nc.sync.dma_start(
                out=bias_qkv[:, :], in_=_ap_with(bq_ap, [[0, 128], *bq_ap.ap])
            )
            bias_proj = cpool.tile([128, C], BF16, tag="bp")
            bp_ap = bp_d[:]
            nc.sync.dma_start(
                out=bias_proj[:, :], in_=_ap_with(bp_ap, [[0, 128], *bp_ap.ap])
            )
        ones_col = cpool.tile([128, 1], BF16, tag="ones1")
        nc.vector.memset(ones_col[:, :], 1.0)

        def load_x(b):
            xhi = xtpool.tile([128, KT, N], F8, tag="xhi", name="xhi")
            nc.sync.dma_start(
                out=xhi[:, :, :], in_=xhi_d[b].rearrange("k p n -> p k n")
            )
            xlo = xtpool.tile([128, KT, N], F8, tag="xlo", name="xlo")
            nc.sync.dma_start(
                out=xlo[:, :, :], in_=xlo_d[b].rearrange("k p n -> p k n")
            )
            return xhi, xlo

        def load_weights(b0):
            # first-consumed slices first: the opening q-chain needs only
            # whi/wlo cols 0:512 and x tile 0 - land those in small DMAs so
            # PE starts early, then stream the rest
            # per-k-pair slices: the opening q-chain starts after ~128KB
            # instead of waiting for the full 1.5MB column block
            nc.sync.dma_start(
                out=whi[:, 0:2, 0:512],
                in_=whi_d[0:2, :, 0:512].rearrange("k p n -> p k n"),
            )
            xhi = xtpool.tile([128, KT, N], F8, tag="xhi", name="xhi")
            nc.sync.dma_start(
                out=xhi[:, :, 0:128], in_=xhi_d[b0, :, :, 0:128].rearrange("k p n -> p k n")
            )
            nc.sync.dma_start(
                out=whi[:, 2:, 0:512],
                in_=whi_d[2:, :, 0:512].rearrange("k p n -> p k n"),
            )
            nc.sync.dma_start(
                out=wlo[:, 0:2, 0:512],
                in_=wlo_d[0:2, :, 0:512].rearrange("k p n -> p k n"),
            )
            nc.sync.dma_start(
                out=wlo[:, 2:, 0:512],
                in_=wlo_d[2:, :, 0:512].rearrange("k p n -> p k n"),
            )
            xlo = xtpool.tile([128, KT, N], F8, tag="xlo", name="xlo")
            nc.sync.dma_start(
                out=xlo[:, :, 0:128], in_=xlo_d[b0, :, :, 0:128].rearrange("k p n -> p k n")
            )
            nc.sync.dma_start(
                out=whi[:, :, 512:1024],
                in_=whi_d[:, :, 512:1024].rearrange("k p n -> p k n"),
            )
            nc.sync.dma_start(
                out=wlo[:, :, 512:1024],
                in_=wlo_d[:, :, 512:1024].rearrange("k p n -> p k n"),
            )
            nc.sync.dma_start(
                out=xhi[:, :, 128:], in_=xhi_d[b0, :, :, 128:].rearrange("k p n -> p k n")
            )
            nc.sync.dma_start(
                out=xlo[:, :, 128:], in_=xlo_d[b0, :, :, 128:].rearrange("k p n -> p k n")
            )
            nc.sync.dma_start(
                out=whi[:, :, 1024:],
                in_=whi_d[:, :, 1024:].rearrange("k p n -> p k n"),
            )
            nc.sync.dma_start(
                out=wlo[:, :, 1024:],
                in_=wlo_d[:, :, 1024:].rearrange("k p n -> p k n"),
            )
            nc.sync.dma_start(
                out=wproj[:, :, :], in_=wproj_d[:, :, :].rearrange("k p n -> p k n")
            )
            return xhi, xlo

        def qkv_mm(ps, xts, col_lo, chunk=None):
            """3-chain fp8 DR product into ps[:, 0:1024].

            Generator when chunk is set: yields between groups of `chunk`
            matmuls so the caller can interleave B-phase slots.
            """
            xhi, xlo = xts
            chains = [(xhi, whi), (xhi, wlo), (xlo, whi)]
            nch = len(chains)
            emitted = 0
            for ci, (xx, ww) in enumerate(chains):
                for kp in range(KP):
                    for half in range(2):
                        nc.tensor.matmul(
                            ps[:, half * 512 : (half + 1) * 512],
                            xx[:, 2 * kp : 2 * kp + 2, :],
                            ww[:, 2 * kp : 2 * kp + 2,
                               col_lo + half * 512 : col_lo + (half + 1) * 512],
                            start=(ci == 0 and kp == 0),
                            stop=(ci == nch - 1 and kp == KP - 1),
                            perf_mode=DR,
                        )
                        emitted += 1
                        if chunk and emitted % chunk == 0:
                            yield

        def qk_pipeline(stage, qi, t):
            """rms norm + rope for q (qi=0) or k (qi=1). stage is the bf16
            SBUF copy of the qkv psum. Returns the t2 tile holding the
            roped+scaled bf16 result (transposed by the caller).

            rope commutes with the rms scale (a per-(p,h) scalar times an
            orthogonal pair-rotation), so the rsqrt multiply is applied
            AFTER the rope add, on the Pool engine."""
            src = stage
            t1 = t1pool.tile([128, 1024], BF16, tag="t1", name="t1")
            sq = t1[:, :]
            nc.vector.tensor_mul(sq, src[:, :], src[:, :])
            sq3 = sq.rearrange("p (h d) -> p h d", d=HD)
            # fold twice before reducing: TensorReduce gets no 2x/4x DVE
            # mode, so shrinking its input is cheaper than reducing wide
            nc.vector.tensor_add(sq3[:, :, 0:32], sq3[:, :, 0:32], sq3[:, :, 32:64])
            nc.vector.tensor_add(sq3[:, :, 0:16], sq3[:, :, 0:16], sq3[:, :, 16:32])
            var = spool.tile([128, H], BF16, tag="var", name="var")
            with nc.allow_low_precision("rms var in bf16 (<0.1% on q)"):
                nc.vector.reduce_sum(
                    var[:, :], sq3[:, :, 0:16], axis=mybir.AxisListType.X
                )
            # rsqrt(var/HD): bf16-bits ln trick + exp-bit-trick + Newton.
            lnv = spool.tile([128, H], F32, tag="lnv", name="lnv")
            nc.vector.tensor_scalar(
                out=lnv[:, :], in0=var[:, :].bitcast(mybir.dt.int16),
                scalar1=-16256 - 128 * 6, scalar2=np.log(2.0) / 128,
                op0=ADD, op1=MULT,
            )
            vv = spool.tile([128, H], F32, tag="vv", name="vv")
            nc.vector.tensor_scalar(
                out=vv[:, :], in0=var[:, :], scalar1=1.0 / HD, scalar2=EPS,
                op0=MULT, op1=ADD,
            )
            r0i = spool.tile([128, H], mybir.dt.int32, tag="r0", name="r0i")
            nc.vector.tensor_scalar(
                out=r0i[:, :], in0=lnv[:, :],
                scalar1=-6051101.6, scalar2=1064866805.0,
                op0=MULT, op1=ADD,
            )
            r0 = r0i[:, :].bitcast(mybir.dt.float32)
            rr = spool.tile([128, H], F32, tag="rr", name="rr")
            e2 = spool.tile([128, H], F32, tag="e2", name="e2")
            cur = r0
            for _ in range(int(os.environ.get("KNEWTON", "2"))):
                nc.vector.tensor_mul(e2[:, :], cur, cur)
                nc.vector.scalar_tensor_tensor(
                    out=e2[:, :], in0=e2[:, :], scalar=-0.5, in1=vv[:, :],
                    op0=MULT, op1=MULT,
                )
                nc.vector.scalar_tensor_tensor(
                    out=rr[:, :], in0=e2[:, :], scalar=1.5, in1=cur,
                    op0=ADD, op1=MULT,
                )
                cur = rr[:, :]

            # rope on the UN-normalized src: t2 = src*C, t1 = swap(src)*S
            ti = 0 if share_tabs else 2 * qi
            ctab = tabs[:, ti + 0, t, :]
            stab = tabs[:, ti + 1, t, :]
            qs3 = src[:, :].rearrange("p (h d) -> p h d", d=HD)
            t13 = t1[:, :].rearrange("p (h d) -> p h d", d=HD)
            nc.vector.tensor_mul(
                t13[:, :, 0:32], qs3[:, :, 32:64], _bcast_mid(stab[:, 0:32], H)
            )
            nc.vector.tensor_mul(
                t13[:, :, 32:64], qs3[:, :, 0:32], _bcast_mid(stab[:, 32:64], H)
            )
            t2 = t2pool.tile([128, 1024], BF16, tag="t2", name="t2")
            nc.vector.tensor_mul(
                t2[:, :].rearrange("p (h d) -> p h d", d=HD), qs3,
                _bcast_mid(ctab, H),
            )
            engadd = nc.gpsimd if (KPOOL & 1) else nc.vector
            engadd.tensor_add(t2[:, :], t2[:, :], t1[:, :])
            engadd.tensor_mul(
                t2[:, :].rearrange("p (h d) -> p h d", d=HD),
                t2[:, :].rearrange("p (h d) -> p h d", d=HD),
                _bcast_last(rr[:, :], HD),
            )
            return t2

        def a_step_gen(xt, qkT, v65, t, chunk=CHUNK, qpool=None):
            """one token tile of phase A as a generator: yields between PE
            chunks so the caller can interleave B-phase slots."""
            qpool = qpool or psQ
            qtag = "S" if qpool is psS else "A"
            xhi, xlo = xt
            xts = (xhi[:, :, t * 128 : (t + 1) * 128],
                   xlo[:, :, t * 128 : (t + 1) * 128])
            stages = []
            for qi in range(2):
                ph = qpool.tile([128, 1024], F32, tag=qtag, name="ps_qk")
                yield from qkv_mm(ph[:, :], xts, qi * 1024, chunk=chunk)
                yield  # let the chain drain so the copy joins its queue ready
                stage = qspool.tile([128, 1024], BF16, tag="stage", name="stage")
                if use_bias:
                    nc.vector.scalar_tensor_tensor(
                        out=stage[:, :], in0=ph[:, :], scalar=1.0,
                        in1=bias_qkv[:, qi * 1024 : (qi + 1) * 1024],
                        op0=MULT, op1=ADD,
                    )
                elif os.environ.get("KSTAGE", "act") == "dve":
                    nc.vector.tensor_copy(stage[:, :], ph[:, :])
                else:
                    nc.scalar.copy(stage[:, :], ph[:, :])
                stages.append(stage)
                if qi == 1:
                    tq = qk_pipeline(stages[0], 0, t)
                yield

            # v: 3-chain DR into one [128, 1024] psum; one ACT copy into v65
            psv = qpool.tile([128, 1024], F32, tag=qtag, name="psv")
            yield from qkv_mm(psv[:, :], xts, 2048, chunk=chunk)
            yield
            v3 = v65[:, t, :].rearrange("p (h e) -> p h e", e=HD)
            pv3 = psv[:, :].rearrange("p (h d) -> p h d", d=HD)
            if use_bias:
                nc.vector.scalar_tensor_tensor(
                    out=v3[:, :, :], in0=pv3, scalar=1.0,
                    in1=bias_qkv[:, 2048:3072].rearrange("p (h d) -> p h d", d=HD),
                    op0=MULT, op1=ADD,
                )
            elif os.environ.get("KVCOPY", "dve") == "dve":
                nc.vector.tensor_copy(v3[:, :, :], pv3)
            else:
                nc.scalar.copy(v3[:, :, :], pv3)
            # q-half transpose (its rope chain completed during the k/v mms)
            nc.sync.dma_start_transpose(
                qkT[:, 0:KT, t * 128 : (t + 1) * 128], tq[:, :]
            )
            tk = qk_pipeline(stages[1], 1, t)
            # the transpose WAITS on the rope tail while holding the SP
            # sequencer - delay its emission so it lands nearly-ready
            for _ in range(int(os.environ.get("KTDELAY", "1"))):
                yield
            nc.sync.dma_start_transpose(
                qkT[:, KT : 2 * KT, t * 128 : (t + 1) * 128], tk[:, :]
            )

        def b_phase(attn4, qkT, v65, weave_gen):
            """slot-pipelined phase B: per (h, jt) slot emit S + (delayed)
            exp + (more delayed) O-octet."""
            pending = []  # (h, jt, pt)
            psos = {}
            psd = psDp.tile([128, H, NT], F32, tag="D", name="psd")
            slot = 0

            def emit_o(h, jt, pt):
                if jt == 0:
                    psos[h] = psO2.tile([128, NT, 64], F32, tag="O", name="ps_o")
                ps_o = psos[h]
                vsl = v65[:, jt, h * HD : (h + 1) * HD]
                for ib in range(NT):
                    ptb = pt[:, ib * 128 : (ib + 1) * 128]
                    nc.tensor.matmul(
                        ps_o[:, ib, :], ptb, vsl,
                        start=(jt == 0 and ib == 0),
                        stop=(jt == NT - 1 and ib == NT - 1),
                        skip_group_check=True,
                    )
                    nc.tensor.matmul(
                        psd[:, h, ib : ib + 1], ptb, ones_col[:, :],
                        start=(h == 0 and jt == 0 and ib == 0),
                        stop=(h == H - 1 and jt == NT - 1 and ib == NT - 1),
                        skip_group_check=True,
                    )
                if jt == NT - 1:
                    nrmq.append([h, psos.pop(h), 0])

            def emit_nrm(h, pso):
                # delayed so the reciprocal's wait (the head's last psd
                # matmul) is satisfied before it reaches the DVE queue head
                rec = recpool.tile([128, NT], BF16, tag="rec", name="rec")
                with nc.allow_low_precision("softmax denom recip bf16"):
                    nc.vector.reciprocal(rec[:, :], psd[:, h, :])
                nc.vector.tensor_mul(
                    attn4[:, :, h, :], pso[:, :, :],
                    _bcast_last(rec[:, :], 64),
                )

            def emit_exp(ps_s, dst):
                nc.scalar.activation(
                    dst, ps_s[:, :],
                    mybir.ActivationFunctionType.Exp, scale=0.125,
                )

            front = int(os.environ.get("KFRONT", "64"))
            nrm_delay = int(os.environ.get("KNRMDELAY", "1"))
            expq = []
            nrmq = []
            for h in range(H):
                base = 64 * (h % 2)
                fb = h // 2
                psl = slice(base, base + 64)
                for jt in range(NT):
                    # pop BEFORE emit_o: the next head's O-octet reuses the
                    # single psO buffer, so the normalize read must be
                    # emitted first (write-after-read emission order)
                    for it in nrmq:
                        it[2] += 1
                    if nrmq and nrmq[0][2] >= nrm_delay:
                        hh, pso, _ = nrmq.pop(0)
                        emit_nrm(hh, pso)
                    if weave_gen is not None:
                        next(weave_gen, None)
                        if slot < front:
                            next(weave_gen, None)
                        if slot < int(os.environ.get("KFRONT3", "10")):
                            next(weave_gen, None)
                    ps_s = psS.tile([128, 1024], F32, tag="S", name="ps_s")
                    for ic in range(2):
                        nc.tensor.matmul(
                            ps_s[:, ic * 512 : (ic + 1) * 512],
                            qkT[psl, 8 + fb, jt * 128 : (jt + 1) * 128],
                            qkT[psl, fb, ic * 512 : (ic + 1) * 512],
                            start=True, stop=True,
                            tile_position=(base, 0),
                        )
                    if len(pending) >= O_DELAY:
                        emit_o(*pending.pop(0))
                    # delay exp emission: when it enters the ACT FIFO its S
                    # psum is already complete, so it never blocks the head
                    pt = ptpool.tile([128, 1024], BF16, tag="pt", name="pt")
                    # NOTE: must stay strictly below O_DELAY (the O-octet
                    # consuming pt[n] must be emitted after exp[n] writes it)
                    expq.append((ps_s, pt[:, :]))
                    if len(expq) > min(int(os.environ.get("KEXPDELAY", "1")),
                                       O_DELAY - 1):
                        emit_exp(*expq.pop(0))
                    pending.append((h, jt, pt))
                    slot += 1
            while expq:
                emit_exp(*expq.pop(0))
            for args in pending:
                emit_o(*args)
            for hh, pso, _ in nrmq:
                emit_nrm(hh, pso)

        def c_gen(attn4, b):
            """phase C as a generator (woven into the next batch's B slots):
            attn4 -> XBAR transpose -> attnT -> proj -> DMA out."""

            # attnT transposes have no waits (attn4 is complete): emit them
            # early so they clear the SP queue before the next batch's
            # rope-gated qkT transposes line up behind them
            def transpose(t):
                att = atpool.tile([128, KT, 128], BF16, tag="at", name="attnT")
                nc.sync.dma_start_transpose(att[:, :, :], attn4[:, t, :, :])
                return att

            atts = [transpose(t) for t in range(min(NT, cfg["at"]))]
            yield
            for t in range(NT):
                att = atts[t]
                if t + cfg["at"] < NT:
                    atts.append(transpose(t + cfg["at"]))
                ps_p = psQ.tile([128, 1024], F32,
                                tag=("S" if psQ is psS else "A"), name="ps_p")
                for half in range(2):
                    for k in range(KT):
                        nc.tensor.matmul(
                            ps_p[:, half * 512 : (half + 1) * 512],
                            att[:, k, :],
                            wproj[:, k, half * 512 : (half + 1) * 512],
                            start=(k == 0), stop=(k == KT - 1),
                        )
                    yield
                yield  # let the proj chain finish before the copy queues
                ostage = opool.tile([128, C], F32, tag="ostage", name="ostage")
                if use_bias:
                    nc.vector.tensor_add(
                        ostage[:, :], ps_p[:, :], bias_proj[:, :]
                    )
                elif os.environ.get("KOCOPY", "dve") == "act":
                    nc.scalar.copy(ostage[:, :], ps_p[:, :])
                elif os.environ.get("KOCOPY", "dve") == "dve":
                    nc.vector.tensor_copy(ostage[:, :], ps_p[:, :])
                else:
                    nc.scalar.copy(ostage[:, 0:512], ps_p[:, 0:512])
                    nc.vector.tensor_copy(ostage[:, 512:1024], ps_p[:, 512:1024])
                # out-DMA queue choice: on ACT its wait (the DVE ostage
                # copy) blocks the exp stream; on SP it sits with the
                # transposes (which land nearly-ready now)
                outq = nc.sync if os.environ.get("KOUTQ", "sp") == "sp" else nc.scalar
                outq.dma_start(out=out_d[b, t], in_=ostage[:, :])
                yield

        def alloc_ab():
            qkT = qkpool.tile([128, 2 * KT, N], BF16, tag="qkT", name="qkT")
            v65 = vpool.tile([128, NT, H * HD], BF16, tag="v65", name="v65")
            return qkT, v65

        reps = int(os.environ.get("KREPEAT", "1"))
        batches = [bb for _ in range(reps) for bb in range(BSH)]

        # prologue: weights + A(b0), two token-tile pipelines interleaved
        xt = load_weights(batches[0])
        tiles = alloc_ab()
        from collections import deque

        _done = object()
        gens = [a_step_gen(xt, tiles[0], tiles[1], t,
                           qpool=(psS if t % 2 else psQ)) for t in range(NT)]
        active = deque(gens[:PROLOG])
        gi = PROLOG
        while active:
            g = active.popleft()
            if next(g, _done) is not _done:
                active.append(g)
            elif gi < NT:
                active.append(gens[gi])
                gi += 1

        from itertools import chain as _ichain

        prev_c = None  # (attn4, b) awaiting phase C
        for bi, b in enumerate(batches):
            qkT, v65 = tiles
            attn4 = a4pool.tile([128, NT, H, HD], BF16, tag="attn4", name="attn4")
            wparts = []
            if prev_c is not None:
                wparts.append(c_gen(*prev_c))
            nxt = batches[bi + 1] if bi + 1 < len(batches) else None
            if nxt is not None:
                xt2 = load_x(nxt)
                tiles2 = alloc_ab()

                _ck = int(os.environ.get("KCHUNK0", "3")) if prev_c is None else CHUNK

                def _weave(_xt=xt2, _tl=tiles2, _ck=_ck):
                    for t in range(NT):
                        yield from a_step_gen(_xt, _tl[0], _tl[1], t, chunk=_ck)

                wparts.append(_weave())
            wg = _ichain(*wparts) if wparts else None
            b_phase(attn4, qkT, v65, wg)
            if wg is not None:
                for _ in wg:
                    pass
            prev_c = (attn4, b)
            if nxt is not None:
                xt, tiles = xt2, tiles2
        for _ in c_gen(*prev_c):
            pass

    nc.compile()
    return nc


_NC = {}


def _get_nc(use_bias: bool = False, share_tabs: bool = False):
    key = (bool(use_bias), bool(share_tabs))
    if key not in _NC:
        _NC[key] = _build_module(*key)
    return _NC[key]


def _rope_tables():
    """cos/sin tables exactly as reference.rope_tables, in float32."""
    grid = int(np.sqrt(N))
    half = HD // 2
    freqs = (1.0 / THETA ** (np.arange(0, half, 2, dtype=np.float32) / half)).astype(
        np.float32
    )
    freqs = np.concatenate([freqs, freqs], axis=0)
    t = np.arange(grid, dtype=np.float32)
    f = np.outer(t, freqs).astype(np.float32)
    fh = np.broadcast_to(f[:, None, :], (grid, grid, half))
    fw = np.broadcast_to(f[None, :, :], (grid, grid, half))
    full = np.concatenate([fh, fw], axis=-1).reshape(-1, HD).astype(np.float32)
    return np.cos(full).astype(np.float32), np.sin(full).astype(np.float32)


def _make_inputs(x, qkv_w, qkv_b, proj_w, proj_b, q_gamma, k_gamma,
                 use_bias=False, share_tabs=False):
    cos, sin = _rope_tables()
    sgn = np.where(np.arange(HD) < HD // 2, -1.0, 1.0).astype(np.float32)
    swap = (np.arange(HD) + HD // 2) % HD

    def fold(gamma):
        c = (cos * gamma[None, :]).astype(np.float32)
        s = (sin * sgn[None, :] * gamma[swap][None, :]).astype(np.float32)
        return c, s

    cq, sq = fold(q_gamma.astype(np.float32))
    if share_tabs:
        stack = [cq, sq]
    else:
        ck, sk = fold(k_gamma.astype(np.float32))
        stack = [cq, sq, ck, sk]
    tabs = np.stack(stack, axis=0).reshape(len(stack), NT, 128, HD).astype(NPBF16)

    ws = (qkv_w.astype(np.float32) * WSCALE).reshape(KT, 128, 3 * C)
    whi = np.ascontiguousarray(ws).astype(NPF8)
    wlo = (ws - whi.astype(np.float32)).astype(NPF8)
    wproj_h = np.ascontiguousarray(
        (proj_w.astype(np.float32) / WSCALE).reshape(KT, 128, C)
    ).astype(NPBF16)

    in_maps = []
    for c in range(N_CORES):
        xc = x[c * BSH : (c + 1) * BSH].astype(np.float32)  # [BSH, N, C]
        xt = np.ascontiguousarray(xc.transpose(0, 2, 1)).reshape(BSH, KT, 128, N)
        xhi = xt.astype(NPF8)
        xlo = (xt - xhi.astype(np.float32)).astype(NPF8)
        m = {
            "xhi": xhi,
            "xlo": xlo,
            "whi": whi,
            "wlo": wlo,
            "wproj": wproj_h,
            "tabs": tabs,
        }
        if use_bias:
            m["bq"] = (qkv_b.astype(np.float32) * WSCALE).astype(NPBF16)
            m["bp"] = proj_b.astype(np.float32).astype(NPBF16)
        in_maps.append(m)
    return in_maps


def _run(in_maps, use_bias=False, share_tabs=False, trace=False, **kwargs):
    nc = _get_nc(use_bias, share_tabs)
    return run_bass_kernel_spmd(
        nc, in_maps, core_ids=list(range(N_CORES)), trace=trace, **kwargs
    )


def kernel(x, qkv_w, qkv_b, proj_w, proj_b, q_gamma, k_gamma):
    x = np.asarray(x)
    qkv_b = np.asarray(qkv_b)
    proj_b = np.asarray(proj_b)
    use_bias = bool(np.any(qkv_b != 0) or np.any(proj_b != 0))
    q_gamma = np.asarray(q_gamma)
    k_gamma = np.asarray(k_gamma)
    share_tabs = bool(np.array_equal(q_gamma, k_gamma))
    in_maps = _make_inputs(
        x, np.asarray(qkv_w), qkv_b, np.asarray(proj_w), proj_b,
        q_gamma, k_gamma, use_bias=use_bias, share_tabs=share_tabs,
    )
    res = _run(in_maps, use_bias=use_bias, share_tabs=share_tabs)
    outs = [res.results[c]["out"].reshape(BSH, NT * 128, C) for c in range(N_CORES)]
    return np.concatenate(outs, axis=0).astype(np.float32)
